# revision 20
# baseline (speedup 1.0000x reference)
"""DenseGNN Trainium2 kernel — batch-parallel over 8 NeuronCores.

Strategy:
  - Core b owns graph b (B=8).  W[b] is pre-transposed on host and kept
    RESIDENT in SBUF (12.6 MB) — every gmul reuses it, so W is read from
    HBM exactly once.
  - Activations h live node-major in SBUF (hT[n, c]); gmul outputs are
    produced channel-major [c, m]; convs that feed back into h compute the
    transposed output directly via matmul operand-role choice.  No
    on-device transposes anywhere.
  - BatchNorm (training mode, stats over batch+nodes) needs cross-core
    reduction: per BN layer we AllGather the per-channel (sum, sumsq)
    partials (a few KB) and reduce locally.
  - Matmuls run as float32r (full PE rate for moving dim >= 256, fp32
    storage, near-fp32 accuracy) accumulating in fp32 PSUM.
  - Channels are blocked into 128-partition slots of (j, cchunk); padded
    slots are exact zeros end-to-end (hT pad cols zeroed, padded params
    zeroed on host), so pads contribute nothing.
"""

import math
import numpy as np

# ---- problem constants (hardcoded; must match the reference) ----
J = 3
NF = 64
NB = 3
IFAC = 4
RED = 0.5
NCLS = 2
B = 8
N = 1024
NT = 8            # node tiles of 128
EPS = 1e-5
HSTRIDE = 384     # per-mtile column stride in hT (max padded C)
CNT = float(B * N)  # BN reduction count

O1 = IFAC * NF    # 256: conv1 output channels


def _ncc(C):
    return (C + 127) // 128


# ---------------------------------------------------------------------------
# Layer schedule (static)
# ---------------------------------------------------------------------------
def _schedule():
    """Returns list of layer dicts describing the whole network."""
    layers = []
    c = NF
    for s in (1, 2, 3):
        for i in range(NB):
            ci = c + NF * i
            layers.append(dict(kind="block", pre=f"s{s}b{i}_", C=ci))
        c += NB * NF
        co = int(math.floor(c * RED))
        layers.append(dict(kind="trans", pre=f"t{s}_", C=c, co=co))
        c = co
    return layers, c  # c = 176 final


LAYERS, CFINAL = _schedule()
NT_LAST = J * _ncc(CFINAL)  # 6


# ---------------------------------------------------------------------------
# Param blob layout + packing
# ---------------------------------------------------------------------------
def _blob_layout():
    """name -> (blob_id, offset, shape); blob 0 = weights (fp32r-consumed),
    blob 1 = small params (fp32). All entries contain "wT" iff weights."""
    layout = {}
    off = [0, 0]

    def add(name, shape):
        n = int(np.prod(shape))
        w = 0 if name.endswith("T") else 1
        layout[name] = (w, off[w], tuple(shape))
        off[w] += n

    add("fwT", (128, J * NF))    # row 0: [fw[:,0], fw[:,1], fw[:,2]]
    add("fb", (128, NF))         # first_b broadcast
    for L in LAYERS:
        pre = L["pre"]
        C = L["C"]
        n_cc = _ncc(C)
        n_t = J * n_cc
        if L["kind"] == "block":
            add(pre + "g1b1", (128, 2 * n_t))
            add(pre + "w1T", (n_t, 128, O1))
            add(pre + "g2b2", (128, 4))         # 256 chans -> 2 tiles
            add(pre + "w2T", (2, 128, NF))
            add(pre + "c2b", (128, NF))
        else:
            co = L["co"]
            add(pre + "gb", (128, 2 * n_t))
            add(pre + "cwT", (n_t, 128, co))
            add(pre + "cb", (128, co))
    add("lwT", (NT_LAST, 128, NCLS))
    add("lb", (128, 1))          # column: row o = last_b[o], rest zero
    return layout, tuple(off)


BLOB_LAYOUT, BLOB_SIZES = _blob_layout()


def _pad_vec_slots(vec, C):
    """[J*C] -> [n_t*128] slot-ordered (slot t = j*n_cc + cc), pads zero."""
    n_cc = _ncc(C)
    n_t = J * n_cc
    out = np.zeros((n_t, 128), np.float32)
    v = np.asarray(vec, np.float32)
    for j in range(J):
        for cc in range(n_cc):
            lo = cc * 128
            hi = min(C, lo + 128)
            if hi > lo:
                out[j * n_cc + cc, : hi - lo] = v[j * C + lo : j * C + hi]
    return out  # [n_t, 128]


def _pad_wT_slots(w, C):
    """w [O, J*C] -> [n_t, 128, O] row slot-ordered, pad rows zero."""
    n_cc = _ncc(C)
    n_t = J * n_cc
    O = w.shape[0]
    out = np.zeros((n_t, 128, O), np.float32)
    w = np.asarray(w, np.float32)
    for j in range(J):
        for cc in range(n_cc):
            lo = cc * 128
            hi = min(C, lo + 128)
            if hi > lo:
                out[j * n_cc + cc, : hi - lo, :] = w[:, j * C + lo : j * C + hi].T
    return out


def _gb_tile(gamma, beta, C):
    """-> [128, 2*n_t]: gamma cols then beta cols (slot-ordered)."""
    g = _pad_vec_slots(gamma, C).T  # [128, n_t]
    b = _pad_vec_slots(beta, C).T
    return np.concatenate([g, b], axis=1)


def _bcast(vec, X):
    v = np.asarray(vec, np.float32).reshape(1, X)
    return np.broadcast_to(v, (128, X)).copy()


def pack_params(params):
    p = {k: np.asarray(v, np.float32) for k, v in params.items()}
    blobs = [np.zeros(BLOB_SIZES[0], np.float32),
             np.zeros(BLOB_SIZES[1], np.float32)]

    def put(name, arr):
        w, off, shape = BLOB_LAYOUT[name]
        arr = np.asarray(arr, np.float32)
        assert arr.shape == shape, (name, arr.shape, shape)
        blobs[w][off : off + arr.size] = arr.ravel()

    fwT = np.zeros((128, J * NF), np.float32)
    fwT[0, :] = p["first_w"].T.ravel()              # j-major [3*64]
    put("fwT", fwT)
    put("fb", _bcast(p["first_b"], NF))
    for L in LAYERS:
        pre = L["pre"]
        C = L["C"]
        if L["kind"] == "block":
            put(pre + "g1b1", _gb_tile(p[pre + "bn1g"], p[pre + "bn1b"], C))
            put(pre + "w1T", _pad_wT_slots(p[pre + "c1w"], C))
            g2 = p[pre + "bn2g"].reshape(2, 128).T  # [128, 2]
            b2 = p[pre + "bn2b"].reshape(2, 128).T
            put(pre + "g2b2", np.concatenate([g2, b2], axis=1))
            put(pre + "w2T", p[pre + "c2w"].T.reshape(2, 128, NF))
            put(pre + "c2b", _bcast(p[pre + "c2b"], NF))
        else:
            co = L["co"]
            put(pre + "gb", _gb_tile(p[pre + "bng"], p[pre + "bnb"], C))
            put(pre + "cwT", _pad_wT_slots(p[pre + "cw"], C))
            put(pre + "cb", _bcast(p[pre + "cb"], co))
    put("lwT", _pad_wT_slots(p["last_w"], CFINAL))
    lb = np.zeros((128, 1), np.float32)
    lb[:NCLS, 0] = p["last_b"]
    put("lb", lb)
    return blobs


# ---------------------------------------------------------------------------
# Bass program
# ---------------------------------------------------------------------------
_PROG = None


def _build_program():
    import concourse.bacc as bacc
    import concourse.mybir as mybir
    from concourse import tile

    F32 = mybir.dt.float32
    F32R = mybir.dt.float32r
    ADD = mybir.AluOpType.add
    MULT = mybir.AluOpType.mult
    X = mybir.AxisListType.X
    AF = mybir.ActivationFunctionType
    RG = [list(range(8))]

    nc = bacc.Bacc(
        "TRN2",
        target_bir_lowering=False,
        debug=False,
        enable_asserts=False,
        num_devices=8,
    )
    wt_d = nc.dram_tensor("wt", [J * NT * 128, N], F32R, kind="ExternalInput").ap()
    xt_d = nc.dram_tensor("xt", [128, NT], F32R, kind="ExternalInput").ap()
    pbw_d = nc.dram_tensor("pbw", [BLOB_SIZES[0]], F32R, kind="ExternalInput").ap()
    pbs_d = nc.dram_tensor("pbs", [BLOB_SIZES[1]], F32, kind="ExternalInput").ap()
    out_d = nc.dram_tensor("out", [NCLS, N], F32, kind="ExternalOutput").ap()

    uid = [0]

    def nm(base):
        uid[0] += 1
        return f"{base}{uid[0]}"

    with tile.TileContext(nc) as tc:
        with (
            tc.tile_pool(name="persist", bufs=1) as pp,
            tc.tile_pool(name="work", bufs=2) as wp,
            tc.tile_pool(name="psum", bufs=1, space="PSUM") as psp,
            tc.tile_pool(name="dram", bufs=1, space="DRAM") as dp,
        ):
            wt_sb = pp.tile([128, J * NT * 1024], F32R, name="wt_sb")
            hT = pp.tile([128, NT * HSTRIDE], F32R, name="hT_sb")
            o_sb = pp.tile([128, 9 * 1024], F32R, name="o_sb")
            z1 = pp.tile([128, 2 * 1024], F32R, name="z1_sb")
            xT = pp.tile([128, NT], F32R, name="xT_sb")
            eps_sb = pp.tile([128, 1], F32, name="eps_sb")
            nc.vector.memset(eps_sb[:], EPS)

            # ---- input DMAs (nt-major so first-needed W slabs land first)
            for ntile in range(NT):
                for j in range(J):
                    s = (j * NT + ntile)
                    nc.sync.dma_start(
                        wt_sb[:, s * 1024 : (s + 1) * 1024],
                        wt_d[s * 128 : (s + 1) * 128, :],
                    )
            nc.sync.dma_start(xT[:], xt_d[:])

            # hT starts fully zero (pad-column invariant).  memset can't
            # target fp32r, so zero via ACT copy with scale=0 (input unread).
            for mt in range(NT):
                sl = hT[:, mt * HSTRIDE : (mt + 1) * HSTRIDE]
                nc.scalar.mul(sl, sl, 0.0)

            def load_pb(name, tag, dtype=None):
                w, off, shape = BLOB_LAYOUT[name]
                pb_d = pbw_d if w == 0 else pbs_d
                t = wp.tile([128, int(np.prod(shape)) // 128],
                            F32R if w == 0 else F32, tag=tag,
                            name=nm(tag))
                if len(shape) == 2:
                    src = pb_d[off : off + int(np.prod(shape))].rearrange(
                        "(p x) -> p x", p=128
                    )
                    nc.sync.dma_start(t[:], src)
                else:
                    kt, _, Xc = shape
                    src = pb_d[off : off + int(np.prod(shape))].rearrange(
                        "(t p x) -> p t x", p=128, x=Xc
                    )
                    nc.sync.dma_start(
                        t[:].rearrange("p (t x) -> p t x", x=Xc), src
                    )
                return t

            # ---- gmul: o[slot(j,cc)] = sum_n W[j].T[n,:] h[:,n] ----------
            def emit_gmul(C, with_stats=True):
                n_cc = _ncc(C)
                n_t = J * n_cc
                if with_stats:
                    sh_sum = wp.tile([128, 2 * n_t], F32, tag="shs", name=nm("shs"))
                    sh_sq = wp.tile([128, 2 * n_t], F32, tag="shq", name=nm("shq"))
                else:
                    sh_sum = sh_sq = None
                for cc in range(n_cc):
                    ps = [
                        psp.tile([128, 512], F32, tag=f"ps{k}", name=nm("gps"))
                        for k in range(6)
                    ]
                    for ntile in range(NT):
                        lhsT = hT[:, ntile * HSTRIDE + cc * 128 :
                                  ntile * HSTRIDE + cc * 128 + 128]
                        for j in range(J):
                            s = j * NT + ntile
                            for mc in range(2):
                                nc.tensor.matmul(
                                    ps[j * 2 + mc][:],
                                    lhsT,
                                    wt_sb[:, s * 1024 + mc * 512 :
                                          s * 1024 + mc * 512 + 512],
                                    start=(ntile == 0),
                                    stop=(ntile == NT - 1),
                                )
                    for j in range(J):
                        t = j * n_cc + cc
                        for mc in range(2):
                            dst = o_sb[:, t * 1024 + mc * 512 : t * 1024 + mc * 512 + 512]
                            if with_stats:
                                nc.vector.tensor_scalar(
                                    dst, ps[j * 2 + mc][:], 1.0, None, MULT, ADD,
                                    accum_out=sh_sum[:, 2 * t + mc : 2 * t + mc + 1],
                                )
                                sq = wp.tile([128, 512], F32, tag="misc", name=nm("sq"))
                                nc.scalar.activation(
                                    sq[:], ps[j * 2 + mc][:], AF.Square,
                                    accum_out=sh_sq[:, 2 * t + mc : 2 * t + mc + 1],
                                )
                            else:
                                nc.vector.tensor_copy(dst, ps[j * 2 + mc][:])
                return sh_sum, sh_sq

            # ---- cross-core stats + BN coefficient computation -----------
            def emit_bn_ab(sh_sum, sh_sq, n_half_per, n_t, gb_name):
                """sh_*: [128, n_half_per*n_t] (col = t*n_half_per + i).
                Returns (a, b) [128, n_t] per-slot scale/bias."""
                gb = load_pb(gb_name, "gb")
                s_loc = wp.tile([128, 2 * n_t], F32, tag="sloc", name=nm("sloc"))
                nc.vector.tensor_reduce(
                    s_loc[:, 0:n_t],
                    sh_sum.rearrange("p (t h) -> p t h", h=n_half_per),
                    axis=X, op=ADD,
                )
                nc.vector.tensor_reduce(
                    s_loc[:, n_t : 2 * n_t],
                    sh_sq.rearrange("p (t h) -> p t h", h=n_half_per),
                    axis=X, op=ADD,
                )
                ag_in = dp.tile([128, 2 * n_t], F32, tag=nm("agi"), name=nm("agin"))
                ag_out = dp.tile([8 * 128, 2 * n_t], F32, tag=nm("ago"),
                                 name=nm("agout"), addr_space="Shared")
                nc.sync.dma_start(ag_in[:], s_loc[:])
                nc.gpsimd.collective_compute(
                    "AllGather", mybir.AluOpType.bypass, replica_groups=RG,
                    ins=[ag_in[:].opt()], outs=[ag_out[:].opt()],
                )
                gath = wp.tile([128, 16 * n_t], F32, tag="gath", name=nm("gath"))
                nc.sync.dma_start(
                    gath[:].rearrange("p (r s) -> p r s", r=8),
                    ag_out[:].rearrange("(r p) s -> p r s", p=128),
                )
                S = wp.tile([128, 2 * n_t], F32, tag="stot", name=nm("stot"))
                nc.vector.tensor_reduce(
                    S[:], gath.rearrange("p (r s) -> p s r", r=8), axis=X, op=ADD
                )
                mean = wp.tile([128, n_t], F32, tag="mean", name=nm("mean"))
                nc.scalar.mul(mean[:], S[:, 0:n_t], 1.0 / CNT)
                var = wp.tile([128, n_t], F32, tag="var", name=nm("var"))
                nc.scalar.mul(var[:], S[:, n_t : 2 * n_t], 1.0 / CNT)
                msq = wp.tile([128, n_t], F32, tag="msq", name=nm("msq"))
                nc.vector.tensor_mul(msq[:], mean[:], mean[:])
                nc.vector.tensor_sub(var[:], var[:], msq[:])
                nc.scalar.activation(var[:], var[:], AF.Sqrt, bias=eps_sb[:, 0:1])
                nc.vector.reciprocal(var[:], var[:])        # var now = rstd
                a = wp.tile([128, n_t], F32, tag="a", name=nm("a"))
                nc.vector.tensor_mul(a[:], gb[:, 0:n_t], var[:])
                b = wp.tile([128, n_t], F32, tag="b", name=nm("b"))
                nc.vector.tensor_mul(b[:], mean[:], a[:])
                nc.vector.tensor_sub(b[:], gb[:, n_t : 2 * n_t], b[:])
                return a, b

            # ================== first layer ==================
            # gmul(x): out[j, m] staged in o_sb row 0, col-block j; then
            # h1T = conv^T accumulated as three K=1 matmuls.
            psx = [psp.tile([128, 512], F32, tag=f"ps{k}", name=nm("psx"))
                   for k in range(6)]
            for ntile in range(NT):
                lhsT = xT[:, ntile : ntile + 1]
                for j in range(J):
                    s = j * NT + ntile
                    for mc in range(2):
                        nc.tensor.matmul(
                            psx[j * 2 + mc][:1, :],
                            lhsT,
                            wt_sb[:, s * 1024 + mc * 512 :
                                  s * 1024 + mc * 512 + 512],
                            start=(ntile == 0),
                            stop=(ntile == NT - 1),
                        )
            for j in range(J):
                for mc in range(2):
                    nc.vector.tensor_copy(
                        o_sb[0:1, j * 1024 + mc * 512 : j * 1024 + mc * 512 + 512],
                        psx[j * 2 + mc][:1, :],
                    )
            fwT = load_pb("fwT", "w2", F32R)         # [128, 64], rows 3+ zero
            fb = load_pb("fb", "bias")
            for mt in range(NT):
                pf = psp.tile([128, 512], F32, tag=f"ps{mt % 6}", name=nm("pf"))
                for j in range(J):
                    nc.tensor.matmul(
                        pf[:, 0:NF],
                        o_sb[0:1, j * 1024 + mt * 128 :
                             j * 1024 + mt * 128 + 128],
                        fwT[0:1, j * NF : (j + 1) * NF],
                        start=(j == 0), stop=(j == J - 1),
                    )
                nc.vector.tensor_add(
                    hT[:, mt * HSTRIDE : mt * HSTRIDE + NF],
                    pf[:, 0:NF], fb[:, 0:NF],
                )

            # ================== main stages ==================
            C = NF
            for L in LAYERS:
                pre = L["pre"]
                C = L["C"]
                n_cc = _ncc(C)
                n_t = J * n_cc
                if L["kind"] == "block":
                    sh_sum, sh_sq = emit_gmul(C)
                    a1, b1 = emit_bn_ab(sh_sum, sh_sq, 2, n_t, pre + "g1b1")
                    for t in range(n_t):
                        nc.scalar.activation(
                            o_sb[:, t * 1024 : (t + 1) * 1024],
                            o_sb[:, t * 1024 : (t + 1) * 1024],
                            AF.Relu, bias=b1[:, t : t + 1], scale=a1[:, t : t + 1],
                        )
                    # conv1 -> psum (256 out channels)
                    w1 = load_pb(pre + "w1T", "bigw", F32R)
                    pz = [psp.tile([128, 512], F32, tag=f"ps{k}", name=nm("pz"))
                          for k in range(4)]
                    sh2s = wp.tile([128, 4], F32, tag="shs2", name=nm("shs2"))
                    sh2q = wp.tile([128, 4], F32, tag="shq2", name=nm("shq2"))
                    for oc in range(2):
                        for mc in range(2):
                            for kt in range(n_t):
                                nc.tensor.matmul(
                                    pz[oc * 2 + mc][:],
                                    w1[:, kt * O1 + oc * 128 :
                                       kt * O1 + oc * 128 + 128],
                                    o_sb[:, kt * 1024 + mc * 512 :
                                         kt * 1024 + mc * 512 + 512],
                                    start=(kt == 0), stop=(kt == n_t - 1),
                                )
                            nc.vector.tensor_reduce(
                                sh2s[:, oc * 2 + mc : oc * 2 + mc + 1],
                                pz[oc * 2 + mc][:], axis=X, op=ADD,
                            )
                            sq = wp.tile([128, 512], F32, tag="misc", name=nm("sq2"))
                            nc.scalar.activation(
                                sq[:], pz[oc * 2 + mc][:], AF.Square,
                                accum_out=sh2q[:, oc * 2 + mc : oc * 2 + mc + 1],
                            )
                    a2, b2 = emit_bn_ab(sh2s, sh2q, 2, 2, pre + "g2b2")
                    for oc in range(2):
                        for mc in range(2):
                            nc.scalar.activation(
                                z1[:, oc * 1024 + mc * 512 : oc * 1024 + mc * 512 + 512],
                                pz[oc * 2 + mc][:],
                                AF.Relu, bias=b2[:, oc : oc + 1],
                                scale=a2[:, oc : oc + 1],
                            )
                    # conv2^T -> append to hT at column C
                    w2 = load_pb(pre + "w2T", "w2", F32R)
                    c2b = load_pb(pre + "c2b", "bias")
                    for mt in range(NT):
                        pc = psp.tile([128, 512], F32, tag=f"ps{4 + mt % 2}",
                                      name=nm("pc"))
                        for oc in range(2):
                            nc.tensor.matmul(
                                pc[:, 0:NF],
                                z1[:, oc * 1024 + mt * 128 :
                                   oc * 1024 + mt * 128 + 128],
                                w2[:, oc * NF : oc * NF + NF],
                                start=(oc == 0), stop=(oc == 1),
                            )
                        nc.vector.tensor_add(
                            hT[:, mt * HSTRIDE + C : mt * HSTRIDE + C + NF],
                            pc[:, 0:NF], c2b[:, 0:NF],
                        )
                else:
                    # transition: h = conv^T(relu(bn(gmul(h))))
                    co = L["co"]
                    sh_sum, sh_sq = emit_gmul(C)
                    a1, b1 = emit_bn_ab(sh_sum, sh_sq, 2, n_t, pre + "gb")
                    for t in range(n_t):
                        nc.scalar.activation(
                            o_sb[:, t * 1024 : (t + 1) * 1024],
                            o_sb[:, t * 1024 : (t + 1) * 1024],
                            AF.Relu, bias=b1[:, t : t + 1], scale=a1[:, t : t + 1],
                        )
                    cw = load_pb(pre + "cwT", "bigw", F32R)
                    cb = load_pb(pre + "cb", "bias")
                    for mt in range(NT):
                        pt = psp.tile([128, 512], F32, tag=f"ps{mt % 6}",
                                      name=nm("pt"))
                        for kt in range(n_t):
                            nc.tensor.matmul(
                                pt[:, 0:co],
                                o_sb[:, kt * 1024 + mt * 128 :
                                     kt * 1024 + mt * 128 + 128],
                                cw[:, kt * co : kt * co + co],
                                start=(kt == 0), stop=(kt == n_t - 1),
                            )
                        nc.vector.tensor_add(
                            hT[:, mt * HSTRIDE : mt * HSTRIDE + co],
                            pt[:, 0:co], cb[:, 0:co],
                        )
                        # restore pad invariant: zero [co, C) (C = old width)
                        stale = hT[:, mt * HSTRIDE + co : mt * HSTRIDE + C]
                        nc.scalar.mul(stale, stale, 0.0)
                    C = co

            # ================== last layer ==================
            # out[2, m] = last_w @ gmul(h)  (channel-major, host transposes)
            emit_gmul(CFINAL, with_stats=False)
            lw = load_pb("lwT", "w2", F32R)
            lb = load_pb("lb", "bias")
            out_sb = wp.tile([128, 1024], F32, tag="misc2", name=nm("out_sb"))
            n_t = NT_LAST
            # channel-major: out[o, n] = sum_k lwT[k, o] * o_sb[k, n]
            for mc in range(2):
                po = psp.tile([128, 512], F32, tag=f"ps{mc}", name=nm("po"))
                for kt in range(n_t):
                    nc.tensor.matmul(
                        po[:NCLS, :],
                        lw[:, kt * NCLS : kt * NCLS + NCLS],
                        o_sb[:, kt * 1024 + mc * 512 :
                             kt * 1024 + mc * 512 + 512],
                        start=(kt == 0), stop=(kt == n_t - 1),
                    )
                nc.vector.tensor_scalar(
                    out_sb[:NCLS, mc * 512 : mc * 512 + 512],
                    po[:NCLS, :], lb[:NCLS, 0:1], None, ADD,
                )
            nc.sync.dma_start(out_d[:, :], out_sb[:NCLS, :])

    nc.compile()
    return nc


def _program():
    global _PROG
    if _PROG is None:
        _PROG = _build_program()
    return _PROG


LAST_RESULTS = None


def kernel(W, x, params):
    from concourse import bass_utils

    W = np.asarray(W, np.float32)
    x = np.asarray(x, np.float32)
    blobs = pack_params(params)
    nc = _program()
    in_maps = []
    for b in range(B):
        wt = np.ascontiguousarray(
            W[b].transpose(0, 2, 1).reshape(J * NT * 128, N)
        )
        xt = np.ascontiguousarray(x[b, 0].reshape(NT, 128).T)
        in_maps.append({"wt": wt, "xt": xt, "pbw": blobs[0], "pbs": blobs[1]})
    res = bass_utils.run_bass_kernel_spmd(nc, in_maps, core_ids=list(range(B)))
    global LAST_RESULTS
    LAST_RESULTS = res
    outs = [res.results[b]["out"] for b in range(B)]  # each [2, N]
    return np.stack(outs).transpose(0, 2, 1).astype(np.float32)
